# revision 21
# baseline (speedup 1.0000x reference)
"""Trainium2 Bass kernel for windowed (inverted-window) attention.

Problem: B=2, T=2048, C=2048, H=16 heads, D=128, WINDOW=512.
  q,k,v = x@Wq, x@Wk, x@Wv  (per-head reshape), RoPE on q,k,
  scores masked so positions INSIDE the causal window are masked out
  (attend only to j>i or j<i-511), softmax, o@Wo.

Sharding: 8 cores = 2 (batch) x 4 (head groups of 4 heads).
Each core computes its batch's 4 heads end-to-end plus a partial
output projection (row-chunk of Wo); host sums the 4 partials per batch.

Phase structure (engine-balanced pipeline):
  A1: K projection for all t-blocks (+RoPE)  -- so attention never waits on K
  A2: Q projection (+RoPE) and V, per t-block
  B:  per (i-block, head): scores -> exp(PSUM,1024-wide) -> mask(gpsimd)
      -> AV + softmax-denominator matmuls -> normalize
  C:  per i-block: output projection, bf16 partials DMA'd out

Matmul operands are bf16 (fp32 PSUM accumulation); output partials bf16.
"""

import sys
import numpy as np

for _p in ("/opt/trn_rl_repo",):
    if _p not in sys.path:
        sys.path.insert(0, _p)

import ml_dtypes  # noqa: E402

# If BASS_TRACE is set in the environment, run_bass_kernel_spmd imports
# antenv.axon_hooks, which this container does not ship. Register a stub
# so tracing degrades gracefully instead of crashing.
try:
    import antenv.axon_hooks  # noqa: F401
except ImportError:
    import types as _types

    _hooks = _types.ModuleType("antenv.axon_hooks")
    _hooks._hook = None
    _hooks.set_axon_ntff_profile_hook = lambda h: setattr(_hooks, "_hook", h)
    _hooks.get_axon_ntff_profile_hook = lambda: _hooks._hook
    sys.modules["antenv.axon_hooks"] = _hooks
    import antenv as _antenv

    _antenv.axon_hooks = _hooks
import concourse.bass as bass  # noqa: E402
import concourse.mybir as mybir  # noqa: E402
from concourse.bacc import Bacc  # noqa: E402
from concourse.tile import TileContext  # noqa: E402
from concourse.bass import ts, ds  # noqa: E402
from concourse.bass_utils import run_bass_kernel_spmd  # noqa: E402

B, T, C, H, D = 2, 2048, 2048, 16, 128
HL = 4                # heads per core
NCORES = 8
WINDOW = 512
ROPE_BASE = 10000.0
TB = 512              # i/t block size (matmul free dim)
NTB = T // TB         # 4
CK = C // 128         # 16 contraction chunks for projections
NTC = T // 128        # 16 j-chunks / t-chunks
MASK_OFF = 512        # master strip offset: off = i0 - j0 + MASK_OFF
MASK_W = 1664
F32 = mybir.dt.float32
BF16 = mybir.dt.bfloat16
AF = mybir.ActivationFunctionType
MM_DT = BF16
NP_MM = ml_dtypes.bfloat16

_NC = None
TRACE = False
LAST_RESULT = None    # BassKernelResults of the most recent run (for test.py)


def build_nc():
    nc = Bacc()
    xT = nc.declare_dram_parameter("xT", [C, T], MM_DT, isOutput=False)
    wq = nc.declare_dram_parameter("wq", [C, HL * D], MM_DT, isOutput=False)
    wk = nc.declare_dram_parameter("wk", [C, HL * D], MM_DT, isOutput=False)
    wv = nc.declare_dram_parameter("wv", [C, HL * D], MM_DT, isOutput=False)
    wo = nc.declare_dram_parameter("wo", [HL * D, C], MM_DT, isOutput=False)
    cosx = nc.declare_dram_parameter("cosx", [128, T], MM_DT, isOutput=False)
    sinx = nc.declare_dram_parameter("sinx", [128, T], MM_DT, isOutput=False)
    maskm = nc.declare_dram_parameter("maskm", [128, MASK_W], MM_DT, isOutput=False)
    out = nc.declare_dram_parameter("out", [T, C], MM_DT, isOutput=True)

    xT_v = xT[:].rearrange("(co p) t -> p co t", p=128)   # [128, 16, T]
    wq_v = wq[:].rearrange("(co p) d -> p co d", p=128)   # [128, 16, 512]
    wk_v = wk[:].rearrange("(co p) d -> p co d", p=128)
    wv_v = wv[:].rearrange("(co p) d -> p co d", p=128)
    wo_v = wo[:].rearrange("(h p) c -> p h c", p=128)     # [128, 4, C]

    scale = float(1.0 / np.sqrt(D))

    with TileContext(nc) as tc:
        with (
            tc.tile_pool(name="res", bufs=1) as res,      # long-lived residents
            tc.tile_pool(name="xbp", bufs=11) as xbp,     # streamed x chunks
            tc.tile_pool(name="ropet", bufs=3) as ropet,
            tc.tile_pool(name="ropes", bufs=3) as ropes,
            tc.tile_pool(name="etp", bufs=6) as etp,
            tc.tile_pool(name="smp", bufs=2) as smp,
            tc.tile_pool(name="zp", bufs=1) as zp,
            tc.tile_pool(name="ocb", bufs=3) as ocb,
        ):
            # ---- B-phase residents ----
            QT = res.tile([128, HL, T], MM_DT)    # q transposed [d, t]
            KT = res.tile([128, HL, T], MM_DT)
            V = res.tile([128, NTC, HL * D], MM_DT)   # v natural [t, hd]
            oT = res.tile([128, HL, T], MM_DT)    # per-head o transposed [d, t]

            def rope_chain(ps, OUTT, h, tb, cosb, sinb):
                # RoPE: out = ps*cos + swap(ps)*sin_signed.  The half-swap is
                # done by two partition-offset scalar copies straight out of
                # PSUM (no DMA hop); the cos product reads PSUM on vector.
                sw = ropes.tile([128, TB], MM_DT, tag="sw")
                nc.scalar.copy(sw[0:64, :], ps[64:128, :])
                nc.vector.tensor_copy(sw[64:128, :], ps[0:64, :])
                raw = ropet.tile([128, TB], MM_DT, tag="raw")
                nc.vector.tensor_mul(raw[:], ps[:], cosb[:, ts(tb, TB)])
                nc.vector.tensor_mul(sw[:], sw[:], sinb[:, ts(tb, TB)])
                nc.vector.tensor_add(OUTT[:, h, ts(tb, TB)], sw[:], raw[:])

            # ============ Phase A: QKV projections + RoPE ============
            with (
                tc.tile_pool(name="wp", bufs=1) as wp,
                tc.tile_pool(name="trig", bufs=1) as trig,
                tc.tile_pool(name="psA", bufs=1, space="PSUM") as psA,
            ):
                wvs, wks, wqs = [], [], []
                cosb = trig.tile([128, T], MM_DT, tag="cos")
                sinb = trig.tile([128, T], MM_DT, tag="sin")
                maskb = res.tile([128, MASK_W], MM_DT)
                wob = res.tile([128, HL, C], MM_DT)
                ones = res.tile([128, 128], MM_DT)
                nc.vector.memset(ones[:], 1.0)

                def v_sweep(tb):
                    psvs = [
                        psA.tile(
                            [128, HL * D], F32, tag=f"pq{tco}", name=f"pv{tb}_{tco}"
                        )
                        for tco in range(NTB)
                    ]
                    for ck in range(CK):
                        if tb == 0 and len(wvs) < CK:
                            wvc = wp.tile(
                                [128, HL * D], MM_DT, tag=f"wv{ck}", name=f"wv{ck}"
                            )
                            nc.sync.dma_start(wvc[:], wv_v[:, ck, :])
                            wvs.append(wvc)
                        xb = xbp.tile([128, TB], MM_DT, tag="xtb", name=f"xv{tb}_{ck}")
                        nc.gpsimd.dma_start(xb[:], xT_v[:, ck, ts(tb, TB)])
                        for tco in range(NTB):
                            nc.tensor.matmul(
                                psvs[tco][:], xb[:, ts(tco, 128)], wvs[ck][:],
                                start=(ck == 0), stop=(ck == CK - 1),
                            )
                    for tco in range(NTB):
                        nc.scalar.copy(V[:, tb * NTB + tco, :], psvs[tco][:])

                def qk_sweep(tb):
                    pqs = [
                        psA.tile([128, TB], F32, tag=f"pq{h}", name=f"pq{tb}_{h}")
                        for h in range(HL)
                    ]
                    pks = [
                        psA.tile([128, TB], F32, tag=f"pk{h}", name=f"pk{tb}_{h}")
                        for h in range(HL)
                    ]
                    for ck in range(CK):
                        if tb == 0 and len(wqs) < CK:
                            wkc = wp.tile(
                                [128, HL * D], MM_DT, tag=f"wk{ck}", name=f"wk{ck}"
                            )
                            nc.sync.dma_start(wkc[:], wk_v[:, ck, :])
                            wqc = wp.tile(
                                [128, HL * D], MM_DT, tag=f"wq{ck}", name=f"wq{ck}"
                            )
                            nc.sync.dma_start(wqc[:], wq_v[:, ck, :])
                            wks.append(wkc)
                            wqs.append(wqc)
                        xb = xbp.tile([128, TB], MM_DT, tag="xtb", name=f"xa{tb}_{ck}")
                        nc.gpsimd.dma_start(xb[:], xT_v[:, ck, ts(tb, TB)])
                        for h in range(HL):
                            nc.tensor.matmul(
                                pqs[h][:], wqs[ck][:, ts(h, D)], xb[:],
                                start=(ck == 0), stop=(ck == CK - 1),
                            )
                            nc.tensor.matmul(
                                pks[h][:], wks[ck][:, ts(h, D)], xb[:],
                                start=(ck == 0), stop=(ck == CK - 1),
                            )
                    if tb == 0:
                        nc.sync.dma_start(cosb[:], cosx[:])
                        nc.sync.dma_start(sinb[:], sinx[:])
                    for h in range(HL):
                        rope_chain(pqs[h], QT, h, tb, cosb, sinb)
                    for h in range(HL):
                        rope_chain(pks[h], KT, h, tb, cosb, sinb)

                for tb in range(NTB):
                    if tb == 1:
                        # deferred resident loads (keep tb0's x prefetch fast)
                        nc.gpsimd.dma_start(maskb[:], maskm[:])
                        nc.gpsimd.dma_start(wob[:], wo_v[:])
                    if tb < NTB - 1:
                        v_sweep(tb)
                        qk_sweep(tb)
                    else:
                        # last block: V after QK so its matmuls cover the
                        # final RoPE chains before attention starts
                        qk_sweep(tb)
                        v_sweep(tb)

            # ======== Phase B: attention; Phase C: output projection ========
            with tc.tile_pool(name="psB", bufs=1, space="PSUM") as psum:
                def live_ranges(c, ib):
                    # query columns not fully masked for key-chunk c
                    I0 = ib * TB
                    flo = max(c * 128 + 127 - I0, 0)
                    fhi = min(c * 128 + 511 - I0, TB - 1)
                    if flo > fhi or fhi - flo + 1 < 192:
                        return [(0, TB - 1)], None
                    live = []
                    if flo > 0:
                        live.append((0, flo - 1))
                    if fhi < TB - 1:
                        live.append((fhi + 1, TB - 1))
                    return live, (flo, fhi)

                def b_iter(ib, h):
                    ets = []
                    for cp in range(NTC // 4):
                        ps = psum.tile(
                            [128, 4, TB], F32, tag="S",
                            name=f"pss{h}_{ib}_{cp}",
                        )
                        for k in range(4):
                            c = 4 * cp + k
                            for lo, hi in live_ranges(c, ib)[0]:
                                nc.tensor.matmul(
                                    ps[:, k, lo:hi + 1], KT[:, h, ts(c, 128)],
                                    QT[:, h, ds(ib * TB + lo, hi - lo + 1)],
                                    start=True, stop=True,
                                )
                        et = etp.tile([128, 4, TB], MM_DT, tag="et")
                        nc.scalar.activation(et[:], ps[:], AF.Exp, scale=scale)
                        for k in range(4):
                            c = 4 * cp + k
                            I0 = ib * TB
                            off = I0 - c * 128 + MASK_OFF
                            # fully-masked column range -> cheap memset
                            flo = max(c * 128 + 127 - I0, 0)
                            fhi = min(c * 128 + 511 - I0, TB - 1)
                            if flo <= fhi:
                                nc.gpsimd.memset(et[:, k, flo:fhi + 1], 0.0)
                            # partially-masked edges -> narrow mask multiply
                            for plo, phi in (
                                (c * 128 - I0, c * 128 + 126 - I0),
                                (c * 128 + 512 - I0, c * 128 + 638 - I0),
                            ):
                                plo, phi = max(plo, 0), min(phi, TB - 1)
                                if plo <= phi:
                                    nc.vector.tensor_mul(
                                        et[:, k, plo:phi + 1],
                                        et[:, k, plo:phi + 1],
                                        maskb[:, ds(off + plo, phi - plo + 1)],
                                    )
                        ets.append(et)
                    pso = psum.tile([128, TB], F32, tag="pso", name=f"po{h}_{ib}")
                    psz = psum.tile([128, TB], F32, tag="psz", name=f"pz{h}_{ib}")
                    us = []
                    for k in range(NTC // 2):
                        u = zp.tile(
                            [128, TB], MM_DT, tag=f"u{k}", name=f"u{h}_{ib}_{k}"
                        )
                        nc.vector.tensor_add(
                            u[:], ets[k // 2][:, 2 * (k % 2), :],
                            ets[k // 2][:, 2 * (k % 2) + 1, :],
                        )
                        us.append(u)
                    for st in (2, 4, 8):
                        for k in range(0, NTC // 2, st):
                            nc.vector.tensor_add(
                                us[k][:], us[k][:], us[k + st // 2][:]
                            )
                    first = True
                    for c in range(NTC):
                        ranges = live_ranges(c, ib)[0]
                        for ri, (lo, hi) in enumerate(ranges):
                            last = c == NTC - 1 and ri == len(ranges) - 1
                            nc.tensor.matmul(
                                pso[:, lo:hi + 1], V[:, c, ts(h, D)],
                                ets[c // 4][:, c % 4, lo:hi + 1],
                                start=first, stop=last,
                            )
                            first = False
                        if c < 1:
                            nc.tensor.matmul(
                                psz[:], ones[:], us[0][:],
                                start=True, stop=True,
                            )
                    rz = smp.tile([128, TB], F32, tag="rz")
                    nc.vector.reciprocal_approx_fast(rz[:], psz[:])
                    nc.vector.tensor_mul(oT[:, h, ts(ib, TB)], pso[:], rz[:])

                def c_chunk(ib, cb):
                    # output projection for column block cb of i-block ib
                    for tto in range(NTB):
                        tt = ib * NTB + tto
                        ps = psum.tile(
                            [128, TB], F32, tag=f"oc{tto % 2}",
                            name=f"psc{ib}_{cb}_{tto}",
                        )
                        for h in range(HL):
                            nc.tensor.matmul(
                                ps[:], oT[:, h, ts(tt, 128)],
                                wob[:, h, ts(cb, TB)],
                                start=(h == 0), stop=(h == HL - 1),
                            )
                        ob = ocb.tile([128, TB], MM_DT, tag="ob")
                        if (cb + tto) % 2 == 0:
                            nc.vector.tensor_copy(ob[:], ps[:])
                            nc.sync.dma_start(out[ts(tt, 128), ts(cb, TB)], ob[:])
                        else:
                            nc.scalar.copy(ob[:], ps[:])
                            nc.scalar.dma_start(out[ts(tt, 128), ts(cb, TB)], ob[:])

                # interleave: C chunks of block ib-1 slot between the head
                # iterations of block ib, filling PE stalls when exp lags
                for ib in range(NTB):
                    for h in range(HL):
                        b_iter(ib, h)
                        if ib > 0:
                            c_chunk(ib - 1, h)
                for cb in range(NTB):
                    c_chunk(NTB - 1, cb)

    nc.finalize()
    return nc


def _host_tables():
    inv_freq = (
        1.0 / (np.float32(ROPE_BASE) ** (np.arange(0, D, 2, dtype=np.float32) / np.float32(D)))
    ).astype(np.float32)
    t = np.arange(T, dtype=np.float32)
    freqs = (t[:, None] * inv_freq[None, :]).astype(np.float32)  # [T, 64]
    cos = np.cos(freqs).T.astype(np.float32)                     # [64, T]
    sin = np.sin(freqs).T.astype(np.float32)
    cosx = np.ascontiguousarray(np.concatenate([cos, cos], axis=0).astype(NP_MM))
    sinx = np.ascontiguousarray(np.concatenate([-sin, sin], axis=0).astype(NP_MM))
    p = np.arange(128, dtype=np.int64)[:, None]
    u = np.arange(MASK_W, dtype=np.int64)[None, :]
    delta = u - MASK_OFF - p          # = i - j for tile offset
    allow = ~((delta >= 0) & (delta <= WINDOW - 1))
    maskm = np.ascontiguousarray(allow.astype(NP_MM))
    return cosx, sinx, maskm


def kernel(x, Wq, Wk, Wv, Wo):
    global _NC, LAST_RESULT
    if _NC is None:
        _NC = build_nc()
    x = np.asarray(x, dtype=np.float32)
    Wq = np.asarray(Wq, dtype=np.float32)
    Wk = np.asarray(Wk, dtype=np.float32)
    Wv = np.asarray(Wv, dtype=np.float32)
    Wo = np.asarray(Wo, dtype=np.float32)
    cosx, sinx, maskm = _host_tables()
    in_maps = []
    for core in range(NCORES):
        b, hg = divmod(core, NCORES // B)
        sl = slice(hg * HL * D, (hg + 1) * HL * D)
        in_maps.append(
            {
                "xT": np.ascontiguousarray(x[b].T.astype(NP_MM)),
                "wq": np.ascontiguousarray(Wq[:, sl].astype(NP_MM)),
                "wk": np.ascontiguousarray(Wk[:, sl].astype(NP_MM)),
                "wv": np.ascontiguousarray(Wv[:, sl].astype(NP_MM)),
                "wo": np.ascontiguousarray(Wo[sl, :].astype(NP_MM)),
                "cosx": cosx,
                "sinx": sinx,
                "maskm": maskm,
            }
        )
    res = run_bass_kernel_spmd(_NC, in_maps, list(range(NCORES)), trace=TRACE)
    LAST_RESULT = res
    out = np.zeros((B, T, C), dtype=np.float32)
    for core in range(NCORES):
        b = core // (NCORES // B)
        out[b] += res.results[core]["out"].astype(np.float32)
    return out


# revision 22
# speedup vs baseline: 1.1391x; 1.1391x over previous
"""Trainium2 Bass kernel for windowed (inverted-window) attention.

Problem: B=2, T=2048, C=2048, H=16 heads, D=128, WINDOW=512.
  q,k,v = x@Wq, x@Wk, x@Wv  (per-head reshape), RoPE on q,k,
  scores masked so positions INSIDE the causal window are masked out
  (attend only to j>i or j<i-511), softmax, o@Wo.

Sharding: 8 cores = 2 (batch) x 4 (head groups of 4 heads).
Each core computes its batch's 4 heads end-to-end plus a partial
output projection (row-chunk of Wo); host sums the 4 partials per batch.

Phase structure (engine-balanced pipeline):
  A1: K projection for all t-blocks (+RoPE)  -- so attention never waits on K
  A2: Q projection (+RoPE) and V, per t-block
  B:  per (i-block, head): scores -> exp(PSUM,1024-wide) -> mask(gpsimd)
      -> AV + softmax-denominator matmuls -> normalize
  C:  per i-block: output projection, bf16 partials DMA'd out

Matmul operands are bf16 (fp32 PSUM accumulation); output partials bf16.
"""

import sys
import numpy as np

for _p in ("/opt/trn_rl_repo",):
    if _p not in sys.path:
        sys.path.insert(0, _p)

import ml_dtypes  # noqa: E402

# If BASS_TRACE is set in the environment, run_bass_kernel_spmd imports
# antenv.axon_hooks, which this container does not ship. Register a stub
# so tracing degrades gracefully instead of crashing.
try:
    import antenv.axon_hooks  # noqa: F401
except ImportError:
    import types as _types

    _hooks = _types.ModuleType("antenv.axon_hooks")
    _hooks._hook = None
    _hooks.set_axon_ntff_profile_hook = lambda h: setattr(_hooks, "_hook", h)
    _hooks.get_axon_ntff_profile_hook = lambda: _hooks._hook
    sys.modules["antenv.axon_hooks"] = _hooks
    import antenv as _antenv

    _antenv.axon_hooks = _hooks
import concourse.bass as bass  # noqa: E402
import concourse.mybir as mybir  # noqa: E402
from concourse.bacc import Bacc  # noqa: E402
from concourse.tile import TileContext  # noqa: E402
from concourse.bass import ts, ds  # noqa: E402
from concourse.bass_utils import run_bass_kernel_spmd  # noqa: E402

B, T, C, H, D = 2, 2048, 2048, 16, 128
HL = 4                # heads per core
NCORES = 8
WINDOW = 512
ROPE_BASE = 10000.0
TB = 512              # i/t block size (matmul free dim)
NTB = T // TB         # 4
CK = C // 128         # 16 contraction chunks for projections
NTC = T // 128        # 16 j-chunks / t-chunks
MASK_OFF = 512        # master strip offset: off = i0 - j0 + MASK_OFF
MASK_W = 1664
F32 = mybir.dt.float32
BF16 = mybir.dt.bfloat16
AF = mybir.ActivationFunctionType
MM_DT = BF16
NP_MM = ml_dtypes.bfloat16

_NC = None
TRACE = False
LAST_RESULT = None    # BassKernelResults of the most recent run (for test.py)


def build_nc():
    nc = Bacc()
    xT = nc.declare_dram_parameter("xT", [C, T], MM_DT, isOutput=False)
    wq = nc.declare_dram_parameter("wq", [C, HL * D], MM_DT, isOutput=False)
    wk = nc.declare_dram_parameter("wk", [C, HL * D], MM_DT, isOutput=False)
    wv = nc.declare_dram_parameter("wv", [C, HL * D], MM_DT, isOutput=False)
    wo = nc.declare_dram_parameter("wo", [HL * D, C], MM_DT, isOutput=False)
    cosx = nc.declare_dram_parameter("cosx", [128, T], MM_DT, isOutput=False)
    sinx = nc.declare_dram_parameter("sinx", [128, T], MM_DT, isOutput=False)
    maskm = nc.declare_dram_parameter("maskm", [128, MASK_W], MM_DT, isOutput=False)
    out = nc.declare_dram_parameter("out", [T, C], MM_DT, isOutput=True)

    xT_v = xT[:].rearrange("(co p) t -> p co t", p=128)   # [128, 16, T]
    wq_v = wq[:].rearrange("(co p) d -> p co d", p=128)   # [128, 16, 512]
    wk_v = wk[:].rearrange("(co p) d -> p co d", p=128)
    wv_v = wv[:].rearrange("(co p) d -> p co d", p=128)
    wo_v = wo[:].rearrange("(h p) c -> p h c", p=128)     # [128, 4, C]

    scale = float(1.0 / np.sqrt(D))

    with TileContext(nc) as tc:
        with (
            tc.tile_pool(name="res", bufs=1) as res,      # long-lived residents
            tc.tile_pool(name="xbp", bufs=11) as xbp,     # streamed x chunks
            tc.tile_pool(name="ropet", bufs=3) as ropet,
            tc.tile_pool(name="ropes", bufs=3) as ropes,
            tc.tile_pool(name="etp", bufs=12) as etp,
            tc.tile_pool(name="smp", bufs=2) as smp,
            tc.tile_pool(name="zp", bufs=1) as zp,
            tc.tile_pool(name="ocb", bufs=3) as ocb,
        ):
            # ---- B-phase residents ----
            QT = res.tile([128, HL, T], MM_DT)    # q transposed [d, t]
            KT = res.tile([128, HL, T], MM_DT)
            V = res.tile([128, NTC, HL * D], MM_DT)   # v natural [t, hd]
            oT = res.tile([128, HL, T], MM_DT)    # per-head o transposed [d, t]

            def rope_chain(ps, OUTT, h, tb, cosb, sinb):
                # RoPE: out = ps*cos + swap(ps)*sin_signed.  The half-swap is
                # done by two partition-offset scalar copies straight out of
                # PSUM (no DMA hop); the cos product reads PSUM on vector.
                sw = ropes.tile([128, TB], MM_DT, tag="sw")
                nc.scalar.copy(sw[0:64, :], ps[64:128, :])
                nc.vector.tensor_copy(sw[64:128, :], ps[0:64, :])
                raw = ropet.tile([128, TB], MM_DT, tag="raw")
                nc.vector.tensor_mul(raw[:], ps[:], cosb[:, ts(tb, TB)])
                nc.vector.tensor_mul(sw[:], sw[:], sinb[:, ts(tb, TB)])
                nc.vector.tensor_add(OUTT[:, h, ts(tb, TB)], sw[:], raw[:])

            # ============ Phase A: QKV projections + RoPE ============
            with (
                tc.tile_pool(name="wp", bufs=1) as wp,
                tc.tile_pool(name="trig", bufs=1) as trig,
                tc.tile_pool(name="psA", bufs=1, space="PSUM") as psA,
            ):
                wvs, wks, wqs = [], [], []
                cosb = trig.tile([128, T], MM_DT, tag="cos")
                sinb = trig.tile([128, T], MM_DT, tag="sin")
                maskb = res.tile([128, MASK_W], MM_DT)
                wob = res.tile([128, HL, C], MM_DT)
                ones = res.tile([128, 128], MM_DT)
                nc.vector.memset(ones[:], 1.0)

                def v_sweep(tb):
                    psvs = [
                        psA.tile(
                            [128, HL * D], F32, tag=f"pq{tco}", name=f"pv{tb}_{tco}"
                        )
                        for tco in range(NTB)
                    ]
                    for ck in range(CK):
                        if tb == 0 and len(wvs) < CK:
                            wvc = wp.tile(
                                [128, HL * D], MM_DT, tag=f"wv{ck}", name=f"wv{ck}"
                            )
                            nc.sync.dma_start(wvc[:], wv_v[:, ck, :])
                            wvs.append(wvc)
                        xb = xbp.tile([128, TB], MM_DT, tag="xtb", name=f"xv{tb}_{ck}")
                        nc.gpsimd.dma_start(xb[:], xT_v[:, ck, ts(tb, TB)])
                        for tco in range(NTB):
                            nc.tensor.matmul(
                                psvs[tco][:], xb[:, ts(tco, 128)], wvs[ck][:],
                                start=(ck == 0), stop=(ck == CK - 1),
                            )
                    for tco in range(NTB):
                        nc.scalar.copy(V[:, tb * NTB + tco, :], psvs[tco][:])

                def qk_sweep(tb):
                    pqs = [
                        psA.tile([128, TB], F32, tag=f"pq{h}", name=f"pq{tb}_{h}")
                        for h in range(HL)
                    ]
                    pks = [
                        psA.tile([128, TB], F32, tag=f"pk{h}", name=f"pk{tb}_{h}")
                        for h in range(HL)
                    ]
                    for ck in range(CK):
                        if tb == 0 and len(wqs) < CK:
                            wkc = wp.tile(
                                [128, HL * D], MM_DT, tag=f"wk{ck}", name=f"wk{ck}"
                            )
                            nc.sync.dma_start(wkc[:], wk_v[:, ck, :])
                            wqc = wp.tile(
                                [128, HL * D], MM_DT, tag=f"wq{ck}", name=f"wq{ck}"
                            )
                            nc.sync.dma_start(wqc[:], wq_v[:, ck, :])
                            wks.append(wkc)
                            wqs.append(wqc)
                        xb = xbp.tile([128, TB], MM_DT, tag="xtb", name=f"xa{tb}_{ck}")
                        nc.gpsimd.dma_start(xb[:], xT_v[:, ck, ts(tb, TB)])
                        for h in range(HL):
                            nc.tensor.matmul(
                                pqs[h][:], wqs[ck][:, ts(h, D)], xb[:],
                                start=(ck == 0), stop=(ck == CK - 1),
                            )
                            nc.tensor.matmul(
                                pks[h][:], wks[ck][:, ts(h, D)], xb[:],
                                start=(ck == 0), stop=(ck == CK - 1),
                            )
                    if tb == 0:
                        nc.sync.dma_start(cosb[:], cosx[:])
                        nc.sync.dma_start(sinb[:], sinx[:])
                    for h in range(HL):
                        rope_chain(pqs[h], QT, h, tb, cosb, sinb)
                    for h in range(HL):
                        rope_chain(pks[h], KT, h, tb, cosb, sinb)

                for tb in range(NTB):
                    if tb == 1:
                        # deferred resident loads (keep tb0's x prefetch fast)
                        nc.gpsimd.dma_start(maskb[:], maskm[:])
                        nc.gpsimd.dma_start(wob[:], wo_v[:])
                    if tb < NTB - 1:
                        v_sweep(tb)
                        qk_sweep(tb)
                    else:
                        # last block: V after QK so its matmuls cover the
                        # final RoPE chains before attention starts
                        qk_sweep(tb)
                        v_sweep(tb)

            # ======== Phase B: attention; Phase C: output projection ========
            with tc.tile_pool(name="psB", bufs=1, space="PSUM") as psum:
                def live_ranges(c, ib):
                    # query columns not fully masked for key-chunk c
                    I0 = ib * TB
                    flo = max(c * 128 + 127 - I0, 0)
                    fhi = min(c * 128 + 511 - I0, TB - 1)
                    if flo > fhi or fhi - flo + 1 < 192:
                        return [(0, TB - 1)], None
                    live = []
                    if flo > 0:
                        live.append((0, flo - 1))
                    if fhi < TB - 1:
                        live.append((fhi + 1, TB - 1))
                    return live, (flo, fhi)

                def b_iter(ib, h):
                    ets = []
                    for cp in range(NTC // 2):
                        ps = psum.tile(
                            [128, 2, TB], F32, tag=f"S{cp % 2}",
                            name=f"pss{h}_{ib}_{cp}",
                        )
                        for k in range(2):
                            c = 2 * cp + k
                            for lo, hi in live_ranges(c, ib)[0]:
                                nc.tensor.matmul(
                                    ps[:, k, lo:hi + 1], KT[:, h, ts(c, 128)],
                                    QT[:, h, ds(ib * TB + lo, hi - lo + 1)],
                                    start=True, stop=True,
                                )
                        et = etp.tile([128, 2, TB], MM_DT, tag="et")
                        nc.scalar.activation(et[:], ps[:], AF.Exp, scale=scale)
                        for k in range(2):
                            c = 2 * cp + k
                            I0 = ib * TB
                            off = I0 - c * 128 + MASK_OFF
                            # fully-masked column range -> cheap memset
                            flo = max(c * 128 + 127 - I0, 0)
                            fhi = min(c * 128 + 511 - I0, TB - 1)
                            if flo <= fhi:
                                nc.gpsimd.memset(et[:, k, flo:fhi + 1], 0.0)
                            # partially-masked edges -> narrow mask multiply
                            for plo, phi in (
                                (c * 128 - I0, c * 128 + 126 - I0),
                                (c * 128 + 512 - I0, c * 128 + 638 - I0),
                            ):
                                plo, phi = max(plo, 0), min(phi, TB - 1)
                                if plo <= phi:
                                    nc.vector.tensor_mul(
                                        et[:, k, plo:phi + 1],
                                        et[:, k, plo:phi + 1],
                                        maskb[:, ds(off + plo, phi - plo + 1)],
                                    )
                        ets.append(et)
                    pso = psum.tile([128, TB], F32, tag="pso", name=f"po{h}_{ib}")
                    psz = psum.tile([128, TB], F32, tag="psz", name=f"pz{h}_{ib}")
                    us = []
                    for k in range(NTC // 2):
                        u = zp.tile(
                            [128, TB], MM_DT, tag=f"u{k}", name=f"u{h}_{ib}_{k}"
                        )
                        nc.vector.tensor_add(u[:], ets[k][:, 0, :], ets[k][:, 1, :])
                        us.append(u)
                    for st in (2, 4, 8):
                        for k in range(0, NTC // 2, st):
                            nc.vector.tensor_add(
                                us[k][:], us[k][:], us[k + st // 2][:]
                            )
                    first = True
                    for c in range(NTC):
                        ranges = live_ranges(c, ib)[0]
                        for ri, (lo, hi) in enumerate(ranges):
                            last = c == NTC - 1 and ri == len(ranges) - 1
                            nc.tensor.matmul(
                                pso[:, lo:hi + 1], V[:, c, ts(h, D)],
                                ets[c // 2][:, c % 2, lo:hi + 1],
                                start=first, stop=last,
                            )
                            first = False
                        if c < 1:
                            nc.tensor.matmul(
                                psz[:], ones[:], us[0][:],
                                start=True, stop=True,
                            )
                    rz = smp.tile([128, TB], F32, tag="rz")
                    nc.vector.reciprocal_approx_fast(rz[:], psz[:])
                    nc.vector.tensor_mul(oT[:, h, ts(ib, TB)], pso[:], rz[:])

                def c_chunk(ib, cb):
                    # output projection for column block cb of i-block ib
                    for tto in range(NTB):
                        tt = ib * NTB + tto
                        ps = psum.tile(
                            [128, TB], F32, tag=f"oc{tto % 2}",
                            name=f"psc{ib}_{cb}_{tto}",
                        )
                        for h in range(HL):
                            nc.tensor.matmul(
                                ps[:], oT[:, h, ts(tt, 128)],
                                wob[:, h, ts(cb, TB)],
                                start=(h == 0), stop=(h == HL - 1),
                            )
                        ob = ocb.tile([128, TB], MM_DT, tag="ob")
                        if (cb + tto) % 2 == 0:
                            nc.vector.tensor_copy(ob[:], ps[:])
                            nc.sync.dma_start(out[ts(tt, 128), ts(cb, TB)], ob[:])
                        else:
                            nc.scalar.copy(ob[:], ps[:])
                            nc.scalar.dma_start(out[ts(tt, 128), ts(cb, TB)], ob[:])

                # interleave: C chunks of block ib-1 slot between the head
                # iterations of block ib, filling PE stalls when exp lags
                for ib in range(NTB):
                    for h in range(HL):
                        b_iter(ib, h)
                        if ib > 0:
                            c_chunk(ib - 1, h)
                for cb in range(NTB):
                    c_chunk(NTB - 1, cb)

    nc.finalize()
    return nc


def _host_tables():
    inv_freq = (
        1.0 / (np.float32(ROPE_BASE) ** (np.arange(0, D, 2, dtype=np.float32) / np.float32(D)))
    ).astype(np.float32)
    t = np.arange(T, dtype=np.float32)
    freqs = (t[:, None] * inv_freq[None, :]).astype(np.float32)  # [T, 64]
    cos = np.cos(freqs).T.astype(np.float32)                     # [64, T]
    sin = np.sin(freqs).T.astype(np.float32)
    cosx = np.ascontiguousarray(np.concatenate([cos, cos], axis=0).astype(NP_MM))
    sinx = np.ascontiguousarray(np.concatenate([-sin, sin], axis=0).astype(NP_MM))
    p = np.arange(128, dtype=np.int64)[:, None]
    u = np.arange(MASK_W, dtype=np.int64)[None, :]
    delta = u - MASK_OFF - p          # = i - j for tile offset
    allow = ~((delta >= 0) & (delta <= WINDOW - 1))
    maskm = np.ascontiguousarray(allow.astype(NP_MM))
    return cosx, sinx, maskm


def kernel(x, Wq, Wk, Wv, Wo):
    global _NC, LAST_RESULT
    if _NC is None:
        _NC = build_nc()
    x = np.asarray(x, dtype=np.float32)
    Wq = np.asarray(Wq, dtype=np.float32)
    Wk = np.asarray(Wk, dtype=np.float32)
    Wv = np.asarray(Wv, dtype=np.float32)
    Wo = np.asarray(Wo, dtype=np.float32)
    cosx, sinx, maskm = _host_tables()
    in_maps = []
    for core in range(NCORES):
        b, hg = divmod(core, NCORES // B)
        sl = slice(hg * HL * D, (hg + 1) * HL * D)
        in_maps.append(
            {
                "xT": np.ascontiguousarray(x[b].T.astype(NP_MM)),
                "wq": np.ascontiguousarray(Wq[:, sl].astype(NP_MM)),
                "wk": np.ascontiguousarray(Wk[:, sl].astype(NP_MM)),
                "wv": np.ascontiguousarray(Wv[:, sl].astype(NP_MM)),
                "wo": np.ascontiguousarray(Wo[sl, :].astype(NP_MM)),
                "cosx": cosx,
                "sinx": sinx,
                "maskm": maskm,
            }
        )
    res = run_bass_kernel_spmd(_NC, in_maps, list(range(NCORES)), trace=TRACE)
    LAST_RESULT = res
    out = np.zeros((B, T, C), dtype=np.float32)
    for core in range(NCORES):
        b = core // (NCORES // B)
        out[b] += res.results[core]["out"].astype(np.float32)
    return out
